# revision 5
# baseline (speedup 1.0000x reference)
"""DSA varlen sparse attention for Trainium2, 8 NeuronCores.

Strategy (token-sharded, K/V replicated per core):
  Per core c: tokens t in [c*256, (c+1)*256).
  Instead of gathering 64 K/V rows per token (536 MB of gather traffic),
  compute DENSE per-head scores S^T[j, t] = sum_d K[j,h,d] q[t,h,d] on the
  PE array in bf16, then multiply exp(S^T) by a scattered sparse weight
  matrix tsd^T[j, t] = sum_{k: topk_idx[t,k]=j} topk_scores[t,k]
  (zero elsewhere).  Because softmax's Z cancels in the reference's
  renormalization, the output is exactly
     out[t,h] = (sum_j exp(s[j,t]) * tsd[j,t] * V[j,h]) / (sum_j exp*tsd).
  The sparse scatter is done on-device with GPSIMD local_scatter
  (per-partition scatter, tokens on partitions); duplicate indices are
  pre-merged with a pairwise is_equal/reduce pass on the vector engine.
  The denominator rides as a leading "ones" column of V through the same
  PSUM accumulation.

  Schedule: the dedup->scatter->transpose chain (DVE/GPSIMD) runs
  concurrently with the per-head S^T matmul + exp phase (PE/ACT); the
  mask-multiply and AV matmuls drain behind it.
"""

import numpy as np
import ml_dtypes
from contextlib import ExitStack

T, H, D, DV, TK = 2048, 8, 128, 128, 64
NCORES = 8
TC = T // NCORES          # 256 tokens per core
P = 128
TCH = TC // P             # 2 token chunks of 128
JC = T // P               # 16 key chunks of 128
SCALE = float(D) ** -0.5
HALF = 1024               # local_scatter num_elems limit is < 2048

_CACHE = {}


def _build_program():
    import concourse.mybir as mybir
    import concourse.tile as tile
    from concourse import bacc

    dt = mybir.dt
    Alu = mybir.AluOpType
    Act = mybir.ActivationFunctionType
    Ax = mybir.AxisListType

    nc = bacc.Bacc(None, target_bir_lowering=False, debug=False)
    names = {}
    with ExitStack() as ctx:
        tc = ctx.enter_context(tile.TileContext(nc))
        dram = ctx.enter_context(tc.tile_pool(name="dram", bufs=1, space="DRAM"))
        sb = ctx.enter_context(tc.tile_pool(name="sb", bufs=1))
        pT_pool = ctx.enter_context(tc.tile_pool(name="pTp", bufs=5))
        sm = ctx.enter_context(tc.tile_pool(name="sm", bufs=1))
        sm2 = ctx.enter_context(tc.tile_pool(name="sm2", bufs=2))
        sps = ctx.enter_context(tc.tile_pool(name="spsum", bufs=1, space="PSUM"))
        ops = ctx.enter_context(tc.tile_pool(name="opsum", bufs=4, space="PSUM"))

        # ---------------- DRAM I/O ----------------
        q_d = dram.tile([P, TCH, H * D], dt.float32, kind="ExternalInput")
        k_d = dram.tile([T, H * D], dt.float32, kind="ExternalInput")
        v_d = dram.tile([P, JC, H * D], dt.float32, kind="ExternalInput")
        idx_d = dram.tile([P, TCH, TK], dt.int16, kind="ExternalInput")
        ts_d = dram.tile([P, TCH, TK], dt.float32, kind="ExternalInput")
        ut_d = dram.tile([P, TK * TK], dt.bfloat16, kind="ExternalInput")
        id_d = dram.tile([P, P], dt.bfloat16, kind="ExternalInput")
        out_d = dram.tile([P, TCH, H * DV], dt.float32, kind="ExternalOutput")
        kbf_d = dram.tile([T, H * D], dt.bfloat16)

        names.update(
            q=q_d.name, k=k_d.name, v=v_d.name, idx=idx_d.name, ts=ts_d.name,
            ut=ut_d.name, ident=id_d.name, out=out_d.name,
        )

        # ---------------- SBUF persistent ----------------
        kT = sb.tile([P, H, T], dt.bfloat16, tag="kT")                 # 32KB/p
        vE = sb.tile([P, JC, H, 1 + DV], dt.bfloat16, tag="vE")        # 33KB/p
        qbf = sb.tile([P, TCH, H * D], dt.bfloat16, tag="qbf")
        qT = sb.tile([P, H, TC], dt.bfloat16, tag="qT")
        tsd = sb.tile([P, TCH, T], dt.bfloat16, tag="tsd")
        tsdT = sb.tile([P, JC, TC], dt.bfloat16, tag="tsdT")
        ut = sb.tile([P, TK * TK], dt.bfloat16, tag="ut")
        ident = sb.tile([P, P], dt.bfloat16, tag="ident")
        idx16 = sb.tile([P, TCH, TK], dt.int16, tag="idx16")
        tsf = sb.tile([P, TCH, TK], dt.float32, tag="tsf")
        outs = sb.tile([P, TCH, H * DV], dt.float32, tag="outs")

        # ---------------- loads (small first) ----------------
        nc.sync.dma_start(out=idx16[:], in_=idx_d[:])
        nc.sync.dma_start(out=tsf[:], in_=ts_d[:])
        nc.sync.dma_start(out=ut[:], in_=ut_d[:])
        nc.sync.dma_start(out=ident[:], in_=id_d[:])
        nc.gpsimd.dma_start(out=qbf[:], in_=q_d[:])  # f32 -> bf16 cast DMA

        # K: cast to bf16 in HBM per head, then xbar-transpose into SBUF.
        for h in range(H):
            nc.gpsimd.dma_start(
                out=kbf_d[:, h * D : (h + 1) * D], in_=k_d[:, h * D : (h + 1) * D]
            )
            nc.sync.dma_start_transpose(
                out=kT[:, h, :], in_=kbf_d[:, h * D : (h + 1) * D]
            )
        # V: ones column 0, then two split cast DMAs (descriptor limit).
        nc.vector.memset(vE[:, :, :, 0:1], 1.0)
        for half in range(2):
            cs = slice(half * (JC // 2), (half + 1) * (JC // 2))
            nc.gpsimd.dma_start(
                out=vE[:, cs, :, 1 : 1 + DV],
                in_=v_d[:, cs, :].rearrange("p c (h d) -> p c h d", h=H),
            )

        # ---------------- q transposes: qT[d, h, t] ----------------
        for h in range(H):
            for t in range(TCH):
                ps = ops.tile([P, P], dt.bfloat16, tag="op")
                nc.tensor.transpose(
                    out=ps[:], in_=qbf[:, t, h * D : (h + 1) * D], identity=ident[:]
                )
                nc.scalar.copy(out=qT[:, h, t * P : (t + 1) * P], in_=ps[:])

        # ---------------- dedup + scatter (per token chunk) ----------------
        for t in range(TCH):
            idxf = sm2.tile([P, TK], dt.float32, tag="idxf")
            nc.vector.tensor_copy(out=idxf[:], in_=idx16[:, t, :])
            tsbf = sm2.tile([P, TK], dt.bfloat16, tag="tsbf")
            nc.vector.tensor_copy(out=tsbf[:], in_=tsf[:, t, :])

            eq = sm.tile([P, TK, TK], dt.bfloat16, tag="eq")
            nc.vector.tensor_tensor(
                out=eq[:],
                in0=idxf[:, :, None].to_broadcast([P, TK, TK]),
                in1=idxf[:, None, :].to_broadcast([P, TK, TK]),
                op=Alu.is_equal,
            )
            # ts'_k = sum_{k'} eq * ts_{k'}   (full duplicate-group sum)
            m = sm.tile([P, TK, TK], dt.bfloat16, tag="m")
            nc.vector.tensor_tensor(
                out=m[:], in0=eq[:],
                in1=tsbf[:, None, :].to_broadcast([P, TK, TK]), op=Alu.mult,
            )
            tsum = sm2.tile([P, TK], dt.float32, tag="tsum")
            nc.vector.tensor_reduce(out=tsum[:], in_=m[:], axis=Ax.X, op=Alu.add)
            # u_k = max_{k'>k} eq  -> keep only last occurrence (u == 0)
            nc.vector.tensor_tensor(
                out=eq[:], in0=eq[:],
                in1=ut[:].rearrange("p (a b) -> p a b", a=TK), op=Alu.mult,
            )
            u = sm2.tile([P, TK], dt.bfloat16, tag="u")
            nc.vector.tensor_reduce(out=u[:], in_=eq[:], axis=Ax.X, op=Alu.max)
            keep = sm2.tile([P, TK], dt.float32, tag="keep")
            nc.vector.tensor_scalar(
                out=keep[:], in0=u[:], scalar1=0.0, scalar2=None, op0=Alu.is_equal
            )
            # idxk = (idx + 1) * keep - 1   (-1 marks dropped duplicates)
            a = sm2.tile([P, TK], dt.float32, tag="a")
            nc.vector.tensor_scalar_add(out=a[:], in0=idxf[:], scalar1=1.0)
            b = sm2.tile([P, TK], dt.float32, tag="b")
            nc.vector.tensor_tensor(out=b[:], in0=a[:], in1=keep[:], op=Alu.mult)
            idxk = sm2.tile([P, TK], dt.float32, tag="idxk")
            nc.vector.tensor_scalar_add(out=idxk[:], in0=b[:], scalar1=-1.0)

            # low half: idx if idx < 1024 else -1
            mlo = sm2.tile([P, TK], dt.float32, tag="mlo")
            nc.vector.tensor_scalar(
                out=mlo[:], in0=idxk[:], scalar1=float(HALF), scalar2=None,
                op0=Alu.is_lt,
            )
            c1 = sm2.tile([P, TK], dt.float32, tag="c1")
            nc.vector.tensor_scalar_add(out=c1[:], in0=idxk[:], scalar1=1.0)
            c2 = sm2.tile([P, TK], dt.float32, tag="c2")
            nc.vector.tensor_tensor(out=c2[:], in0=c1[:], in1=mlo[:], op=Alu.mult)
            c3 = sm2.tile([P, TK], dt.float32, tag="c3")
            nc.vector.tensor_scalar_add(out=c3[:], in0=c2[:], scalar1=-1.0)
            ilo = sm2.tile([P, TK], dt.int16, tag="ilo")
            nc.vector.tensor_copy(out=ilo[:], in_=c3[:])
            # high half: idx - 1024 if idx >= 1024 else -1
            mhi = sm2.tile([P, TK], dt.float32, tag="mhi")
            nc.vector.tensor_scalar(
                out=mhi[:], in0=idxk[:], scalar1=float(HALF) - 0.5, scalar2=None,
                op0=Alu.is_gt,
            )
            d1 = sm2.tile([P, TK], dt.float32, tag="d1")
            nc.vector.tensor_scalar_add(out=d1[:], in0=idxk[:], scalar1=1.0 - HALF)
            d2 = sm2.tile([P, TK], dt.float32, tag="d2")
            nc.vector.tensor_tensor(out=d2[:], in0=d1[:], in1=mhi[:], op=Alu.mult)
            d3 = sm2.tile([P, TK], dt.float32, tag="d3")
            nc.vector.tensor_scalar_add(out=d3[:], in0=d2[:], scalar1=-1.0)
            ihi = sm2.tile([P, TK], dt.int16, tag="ihi")
            nc.vector.tensor_copy(out=ihi[:], in_=d3[:])

            tsumbf = sm2.tile([P, TK], dt.bfloat16, tag="tsumbf")
            nc.vector.tensor_copy(out=tsumbf[:], in_=tsum[:])

            nc.gpsimd.local_scatter(
                out_ap=tsd[:, t, 0:HALF], data_ap=tsumbf[:], idxs_ap=ilo[:],
                channels=P, num_elems=HALF, num_idxs=TK,
            )
            nc.gpsimd.local_scatter(
                out_ap=tsd[:, t, HALF:T], data_ap=tsumbf[:], idxs_ap=ihi[:],
                channels=P, num_elems=HALF, num_idxs=TK,
            )

        # ---------------- tsd transpose: tsdT[j, t] (drain on DVE) --------
        for t in range(TCH):
            for jc in range(JC):
                ps = ops.tile([P, P], dt.bfloat16, tag="op")
                nc.tensor.transpose(
                    out=ps[:], in_=tsd[:, t, jc * P : (jc + 1) * P], identity=ident[:]
                )
                nc.vector.tensor_copy(out=tsdT[:, jc, t * P : (t + 1) * P], in_=ps[:])

        # ---------------- phase A: S^T + exp for every head ----------------
        G = 8  # score chunks per PSUM tile (4 banks)
        pTs = []
        for h in range(H):
            pT = pT_pool.tile([P, JC, TC], dt.bfloat16, tag="pT")
            pTs.append(pT)
            for g in range(JC // G):
                sp = sps.tile([P, G, TC], dt.float32, tag="sp")
                for j in range(G):
                    jc = g * G + j
                    nc.tensor.matmul(
                        out=sp[:, j, :],
                        lhsT=kT[:, h, jc * P : (jc + 1) * P],
                        rhs=qT[:, h, :],
                        start=True, stop=True,
                    )
                nc.scalar.activation(
                    out=pT[:, g * G : (g + 1) * G, :], in_=sp[:],
                    func=Act.Exp, scale=SCALE,
                )

        # ---------------- phase B: mask + AV + normalize ----------------
        for h in range(H):
            pT = pTs[h]
            nc.vector.tensor_tensor(
                out=pT[:], in0=pT[:], in1=tsdT[:], op=Alu.mult,
            )
            for t in range(TCH):
                op = ops.tile([P, 1 + DV], dt.float32, tag="op")
                for jc in range(JC):
                    nc.tensor.matmul(
                        out=op[:],
                        lhsT=pT[:, jc, t * P : (t + 1) * P],
                        rhs=vE[:, jc, h, :],
                        start=(jc == 0), stop=(jc == JC - 1),
                    )
                rec = sm2.tile([P, 1], dt.float32, tag="rec")
                nc.vector.reciprocal(out=rec[:], in_=op[:, 0:1])
                nc.scalar.mul(
                    out=outs[:, t, h * DV : (h + 1) * DV], in_=op[:, 1 : 1 + DV],
                    mul=rec[:],
                )

        nc.sync.dma_start(out=out_d[:], in_=outs[:])

    nc.compile()
    return nc, names


def _get_program():
    if "prog" not in _CACHE:
        _CACHE["prog"] = _build_program()
    return _CACHE["prog"]


def _host_inputs(q, k, v, idx, ts):
    """Build per-core in_maps (host-side shard/layout only)."""
    bf16 = ml_dtypes.bfloat16
    # strict upper-triangle (k' > k) replicated over partitions
    utm = np.triu(np.ones((TK, TK), np.float32), 1).reshape(1, TK * TK)
    utm = np.broadcast_to(utm, (P, TK * TK)).astype(bf16)
    identity = np.eye(P, dtype=np.float32).astype(bf16)

    k_full = np.ascontiguousarray(k.reshape(T, H * D).astype(np.float32))
    v_full = (
        v.reshape(JC, P, H * D).transpose(1, 0, 2).astype(np.float32)
    )  # [P, JC, H*D], row j = c*128+p
    v_full = np.ascontiguousarray(v_full)

    maps = []
    for c in range(NCORES):
        sl = slice(c * TC, (c + 1) * TC)
        qc = (
            q[sl].reshape(TCH, P, H * D).transpose(1, 0, 2).astype(np.float32)
        )  # [P, TCH, H*D], t = tc*128 + p
        ic = idx[sl].astype(np.int16).reshape(TCH, P, TK).transpose(1, 0, 2)
        tc_ = ts[sl].astype(np.float32).reshape(TCH, P, TK).transpose(1, 0, 2)
        maps.append(
            dict(
                q=np.ascontiguousarray(qc),
                k=k_full,
                v=v_full,
                idx=np.ascontiguousarray(ic),
                ts=np.ascontiguousarray(tc_),
                ut=utm,
                ident=identity,
            )
        )
    return maps


def kernel(q_packed, k_packed, v_packed, topk_indices, topk_scores):
    from concourse.bass_utils import run_bass_kernel_spmd

    q = np.asarray(q_packed, dtype=np.float32)
    k = np.asarray(k_packed, dtype=np.float32)
    v = np.asarray(v_packed, dtype=np.float32)
    idx = np.asarray(topk_indices)
    ts = np.asarray(topk_scores, dtype=np.float32)

    nc, names = _get_program()
    logical_maps = _host_inputs(q, k, v, idx, ts)
    in_maps = [{names[key]: arr for key, arr in m.items()} for m in logical_maps]

    res = run_bass_kernel_spmd(nc, in_maps, core_ids=list(range(NCORES)))
    outn = names["out"]
    parts = []
    for c in range(NCORES):
        oc = res.results[c][outn]  # [P, TCH, H*DV]
        parts.append(oc.transpose(1, 0, 2).reshape(TC, H, DV))
    return np.concatenate(parts, axis=0).astype(np.float32)


if __name__ == "__main__":
    rng = np.random.default_rng(0)
    q = rng.standard_normal((T, H, D), dtype=np.float32)
    k = rng.standard_normal((T, H, D), dtype=np.float32)
    v = rng.standard_normal((T, H, DV), dtype=np.float32)
    idx = rng.integers(0, T, size=(T, TK), dtype=np.int64)
    ts = rng.random((T, TK), dtype=np.float32)
    out = kernel(q, k, v, idx, ts)
    print(out.shape, out.dtype)


# revision 7
# speedup vs baseline: 1.3372x; 1.3372x over previous
"""DSA varlen sparse attention for Trainium2, 8 NeuronCores.

Strategy (token-sharded, K/V replicated per core):
  Per core c: tokens t in [c*256, (c+1)*256).
  Instead of gathering 64 K/V rows per token (536 MB of gather traffic),
  compute DENSE per-head scores S^T[j, t] = sum_d K[j,h,d] q[t,h,d] on the
  PE array in bf16, then multiply exp(S^T) by a scattered sparse weight
  matrix tsd^T[j, t] = sum_{k: topk_idx[t,k]=j} topk_scores[t,k]
  (zero elsewhere).  Because softmax's Z cancels in the reference's
  renormalization, the output is exactly
     out[t,h] = (sum_j exp(s[j,t]) * tsd[j,t] * V[j,h]) / (sum_j exp*tsd).
  The sparse scatter runs on-device with GPSIMD local_scatter
  (per-partition scatter, tokens on partitions); duplicate indices are
  pre-merged with a pairwise is_equal/reduce pass on the vector engine.
  The denominator rides as a leading "ones" column of V through the same
  PSUM accumulation.

  Engine schedule: DVE does the dedup chain while PE/ACT run per-head
  S^T matmuls + exp (which don't need the mask); GPSIMD scatters, PE
  transposes tsd, then phase B (mask-mul + AV matmuls + normalize)
  drains per (head, token-chunk).  bf16 inputs are prepared host-side
  (layout/sharding prep); all matmul accumulation is fp32 in PSUM.
"""

import numpy as np
import ml_dtypes
from contextlib import ExitStack

T, H, D, DV, TK = 2048, 8, 128, 128, 64
NCORES = 8
TC = T // NCORES          # 256 tokens per core
P = 128
TCH = TC // P             # 2 token chunks of 128
JC = T // P               # 16 key chunks of 128
SCALE = float(D) ** -0.5
HALF = 1024               # local_scatter num_elems limit is < 2048

_CACHE = {}


def _build_program():
    import concourse.mybir as mybir
    import concourse.tile as tile
    from concourse import bacc

    dt = mybir.dt
    Alu = mybir.AluOpType
    Act = mybir.ActivationFunctionType
    Ax = mybir.AxisListType

    nc = bacc.Bacc(None, target_bir_lowering=False, debug=False)
    names = {}
    with ExitStack() as ctx:
        tc = ctx.enter_context(tile.TileContext(nc))
        dram = ctx.enter_context(tc.tile_pool(name="dram", bufs=1, space="DRAM"))
        sb = ctx.enter_context(tc.tile_pool(name="sb", bufs=1))
        pT_pool = ctx.enter_context(tc.tile_pool(name="pTp", bufs=5))
        sm = ctx.enter_context(tc.tile_pool(name="sm", bufs=1))
        sm2 = ctx.enter_context(tc.tile_pool(name="sm2", bufs=2))
        sps = ctx.enter_context(tc.tile_pool(name="spsum", bufs=1, space="PSUM"))
        ops = ctx.enter_context(tc.tile_pool(name="opsum", bufs=4, space="PSUM"))

        # ---------------- DRAM I/O (bf16 data prepped host-side) ----------
        q_d = dram.tile([P, TCH, H * D], dt.bfloat16, kind="ExternalInput")
        k_d = dram.tile([T, H * D], dt.bfloat16, kind="ExternalInput")
        v_d = dram.tile([P, JC * H * (1 + DV)], dt.bfloat16, kind="ExternalInput")
        idx_d = dram.tile([P, TCH, TK], dt.int16, kind="ExternalInput")
        ts_d = dram.tile([P, TCH, TK], dt.bfloat16, kind="ExternalInput")
        ut_d = dram.tile([P, TK * TK], dt.bfloat16, kind="ExternalInput")
        id_d = dram.tile([P, P], dt.bfloat16, kind="ExternalInput")
        out_d = dram.tile([P, TCH, H * DV], dt.float32, kind="ExternalOutput")

        names.update(
            q=q_d.name, k=k_d.name, v=v_d.name, idx=idx_d.name, ts=ts_d.name,
            ut=ut_d.name, ident=id_d.name, out=out_d.name,
        )

        # ---------------- SBUF persistent ----------------
        kT = sb.tile([P, H, T], dt.bfloat16, tag="kT")                 # 32KB/p
        vE = sb.tile([P, JC, H, 1 + DV], dt.bfloat16, tag="vE")        # 33KB/p
        qbf = sb.tile([P, TCH, H * D], dt.bfloat16, tag="qbf")
        qT = sb.tile([P, H, TC], dt.bfloat16, tag="qT")
        tsd = sb.tile([P, TCH, T], dt.bfloat16, tag="tsd")
        tsdT = sb.tile([P, JC, TC], dt.bfloat16, tag="tsdT")
        ut = sb.tile([P, TK * TK], dt.bfloat16, tag="ut")
        ident = sb.tile([P, P], dt.bfloat16, tag="ident")
        idx16 = sb.tile([P, TCH, TK], dt.int16, tag="idx16")
        tsbf = sb.tile([P, TCH, TK], dt.bfloat16, tag="tsbf")
        outs = sb.tile([P, TCH, H * DV], dt.float32, tag="outs")

        # ---------------- loads (small first; all HWDGE) ----------------
        nc.sync.dma_start(out=idx16[:], in_=idx_d[:])
        nc.sync.dma_start(out=tsbf[:], in_=ts_d[:])
        nc.sync.dma_start(out=ident[:], in_=id_d[:])
        # big plain copies ride SWDGE; transposes own the sync HWDGE queue
        # (concurrent HWDGE copy + xbar transpose is a known HW hazard)
        nc.gpsimd.dma_start(out=qbf[:], in_=q_d[:])
        nc.gpsimd.dma_start(out=ut[:], in_=ut_d[:])
        for h in range(H):
            nc.sync.dma_start_transpose(
                out=kT[:, h, :], in_=k_d[:, h * D : (h + 1) * D]
            )
        # V (+ ones column) pre-laid-out on host
        nc.gpsimd.dma_start(
            out=vE[:].rearrange("p a b c -> p (a b c)"), in_=v_d[:]
        )

        # ---------------- dedup chain (DVE), both chunks fused ----------
        idxf = sm2.tile([P, TCH, TK], dt.float32, tag="idxf")
        nc.vector.tensor_copy(out=idxf[:], in_=idx16[:])
        eq = sm.tile([P, TCH, TK, TK], dt.bfloat16, tag="eq")
        nc.vector.tensor_tensor(
            out=eq[:],
            in0=idxf[:, :, :, None].to_broadcast([P, TCH, TK, TK]),
            in1=idxf[:, :, None, :].to_broadcast([P, TCH, TK, TK]),
            op=Alu.is_equal,
        )
        # ts'_k = sum_{k'} eq * ts_{k'}   (full duplicate-group sum)
        m = sm.tile([P, TCH, TK, TK], dt.bfloat16, tag="m")
        nc.vector.tensor_tensor(
            out=m[:], in0=eq[:],
            in1=tsbf[:, :, None, :].to_broadcast([P, TCH, TK, TK]), op=Alu.mult,
        )
        tsum = sm2.tile([P, TCH, TK], dt.bfloat16, tag="tsum")
        with nc.allow_low_precision("duplicate-group sums have <=4 terms"):
            nc.vector.tensor_reduce(out=tsum[:], in_=m[:], axis=Ax.X, op=Alu.add)
        # u_k = max_{k'>k} eq  -> keep only the last occurrence (u == 0)
        nc.vector.tensor_tensor(
            out=eq[:], in0=eq[:],
            in1=ut[:].rearrange("p (a b) -> p a b", a=TK)[:, None, :, :]
            .to_broadcast([P, TCH, TK, TK]),
            op=Alu.mult,
        )
        u = sm2.tile([P, TCH, TK], dt.bfloat16, tag="u")
        nc.vector.tensor_reduce(out=u[:], in_=eq[:], axis=Ax.X, op=Alu.max)
        keep = sm2.tile([P, TCH, TK], dt.float32, tag="keep")
        nc.vector.tensor_scalar(
            out=keep[:], in0=u[:], scalar1=0.0, scalar2=None, op0=Alu.is_equal
        )
        # b = (idx + 1) * keep;  kept slots: b = idx+1 >= 1, dropped: b = 0
        a = sm2.tile([P, TCH, TK], dt.float32, tag="a")
        nc.vector.tensor_scalar_add(out=a[:], in0=idxf[:], scalar1=1.0)
        b = sm2.tile([P, TCH, TK], dt.float32, tag="b")
        nc.vector.tensor_tensor(out=b[:], in0=a[:], in1=keep[:], op=Alu.mult)
        # low half: idx' = b*[b < 1025] - 1  (in [0,1024) or -1)
        mlo = sm2.tile([P, TCH, TK], dt.float32, tag="mlo")
        nc.vector.tensor_scalar(
            out=mlo[:], in0=b[:], scalar1=float(HALF) + 0.5, scalar2=None,
            op0=Alu.is_lt,
        )
        c2 = sm2.tile([P, TCH, TK], dt.float32, tag="c2")
        nc.vector.tensor_tensor(out=c2[:], in0=b[:], in1=mlo[:], op=Alu.mult)
        ilo = sm2.tile([P, TCH, TK], dt.int16, tag="ilo")
        nc.vector.tensor_scalar_add(out=ilo[:], in0=c2[:], scalar1=-1.0)
        # high half: idx' = (b - 1024)*[b > 1024] - 1
        mhi = sm2.tile([P, TCH, TK], dt.float32, tag="mhi")
        nc.vector.tensor_scalar(
            out=mhi[:], in0=b[:], scalar1=float(HALF) + 0.5, scalar2=None,
            op0=Alu.is_gt,
        )
        d1 = sm2.tile([P, TCH, TK], dt.float32, tag="d1")
        nc.vector.tensor_scalar_add(out=d1[:], in0=b[:], scalar1=-float(HALF))
        d2 = sm2.tile([P, TCH, TK], dt.float32, tag="d2")
        nc.vector.tensor_tensor(out=d2[:], in0=d1[:], in1=mhi[:], op=Alu.mult)
        ihi = sm2.tile([P, TCH, TK], dt.int16, tag="ihi")
        nc.vector.tensor_scalar_add(out=ihi[:], in0=d2[:], scalar1=-1.0)

        for t in range(TCH):
            nc.gpsimd.local_scatter(
                out_ap=tsd[:, t, 0:HALF], data_ap=tsum[:, t, :], idxs_ap=ilo[:, t, :],
                channels=P, num_elems=HALF, num_idxs=TK,
            )
            nc.gpsimd.local_scatter(
                out_ap=tsd[:, t, HALF:T], data_ap=tsum[:, t, :], idxs_ap=ihi[:, t, :],
                channels=P, num_elems=HALF, num_idxs=TK,
            )

        # ---------------- q transposes: qT[d, h, t] (PE, early) ----------
        for h in range(H):
            for t in range(TCH):
                ps = ops.tile([P, P], dt.bfloat16, tag="op")
                nc.tensor.transpose(
                    out=ps[:], in_=qbf[:, t, h * D : (h + 1) * D], identity=ident[:]
                )
                nc.scalar.copy(out=qT[:, h, t * P : (t + 1) * P], in_=ps[:])

        # ---------------- phase A: S^T + exp for every head ----------------
        G = 8  # score chunks per PSUM tile (4 banks)
        pTs = []
        for h in range(H):
            pT = pT_pool.tile([P, JC, TC], dt.bfloat16, tag="pT")
            pTs.append(pT)
            for g in range(JC // G):
                sp = sps.tile([P, G, TC], dt.float32, tag="sp")
                for j in range(G):
                    jc = g * G + j
                    nc.tensor.matmul(
                        out=sp[:, j, :],
                        lhsT=kT[:, h, jc * P : (jc + 1) * P],
                        rhs=qT[:, h, :],
                        start=True, stop=True,
                    )
                nc.scalar.activation(
                    out=pT[:, g * G : (g + 1) * G, :], in_=sp[:],
                    func=Act.Exp, scale=SCALE,
                )

        # ------------- tsd transpose: tsdT[j, t] (PE after phase A) ------
        # chunk-0 drains on ACT, chunk-1 drains on DVE, so phase B's
        # t=0 work can start while chunk 1 still drains.
        for t in range(TCH):
            for jc in range(JC):
                ps = ops.tile([P, P], dt.bfloat16, tag="op")
                nc.tensor.transpose(
                    out=ps[:], in_=tsd[:, t, jc * P : (jc + 1) * P], identity=ident[:]
                )
                dst = tsdT[:, jc, t * P : (t + 1) * P]
                if t == 0:
                    nc.scalar.copy(out=dst, in_=ps[:])
                else:
                    nc.vector.tensor_copy(out=dst, in_=ps[:])

        # ---------------- phase B: mask + AV + normalize ----------------
        for h in range(H):
            pT = pTs[h]
            for t in range(TCH):
                nc.vector.tensor_tensor(
                    out=pT[:, :, t * P : (t + 1) * P],
                    in0=pT[:, :, t * P : (t + 1) * P],
                    in1=tsdT[:, :, t * P : (t + 1) * P],
                    op=Alu.mult,
                )
                op = ops.tile([P, 1 + DV], dt.float32, tag="op")
                for jc in range(JC):
                    nc.tensor.matmul(
                        out=op[:],
                        lhsT=pT[:, jc, t * P : (t + 1) * P],
                        rhs=vE[:, jc, h, :],
                        start=(jc == 0), stop=(jc == JC - 1),
                    )
                rec = sm2.tile([P, 1], dt.float32, tag="rec")
                nc.vector.reciprocal(out=rec[:], in_=op[:, 0:1])
                nc.scalar.mul(
                    out=outs[:, t, h * DV : (h + 1) * DV], in_=op[:, 1 : 1 + DV],
                    mul=rec[:],
                )

        for t in range(TCH):
            nc.sync.dma_start(out=out_d[:, t, :], in_=outs[:, t, :])

    nc.compile()
    return nc, names


def _get_program():
    if "prog" not in _CACHE:
        _CACHE["prog"] = _build_program()
    return _CACHE["prog"]


def _host_inputs(q, k, v, idx, ts):
    """Build per-core in_maps (host-side shard/layout/dtype prep)."""
    bf16 = ml_dtypes.bfloat16
    # strict upper-triangle (k' > k) replicated over partitions
    utm = np.triu(np.ones((TK, TK), np.float32), 1).reshape(1, TK * TK)
    utm = np.ascontiguousarray(np.broadcast_to(utm, (P, TK * TK))).astype(bf16)
    identity = np.eye(P, dtype=np.float32).astype(bf16)

    k_full = np.ascontiguousarray(k.reshape(T, H * D)).astype(bf16)
    # vE[p, jc, h, 0] = 1, vE[p, jc, h, 1:] = V[jc*128+p, h, :]
    v_r = v.reshape(JC, P, H, DV).transpose(1, 0, 2, 3)  # [P, JC, H, DV]
    v_full = np.ones((P, JC, H, 1 + DV), dtype=np.float32)
    v_full[:, :, :, 1:] = v_r
    v_full = v_full.reshape(P, JC * H * (1 + DV)).astype(bf16)

    maps = []
    for c in range(NCORES):
        sl = slice(c * TC, (c + 1) * TC)
        qc = (
            q[sl].reshape(TCH, P, H * D).transpose(1, 0, 2)
        )  # [P, TCH, H*D], t = tc*128 + p
        ic = idx[sl].astype(np.int16).reshape(TCH, P, TK).transpose(1, 0, 2)
        tc_ = ts[sl].reshape(TCH, P, TK).transpose(1, 0, 2)
        maps.append(
            dict(
                q=np.ascontiguousarray(qc).astype(bf16),
                k=k_full,
                v=v_full,
                idx=np.ascontiguousarray(ic),
                ts=np.ascontiguousarray(tc_).astype(bf16),
                ut=utm,
                ident=identity,
            )
        )
    return maps


def kernel(q_packed, k_packed, v_packed, topk_indices, topk_scores):
    from concourse.bass_utils import run_bass_kernel_spmd

    q = np.asarray(q_packed, dtype=np.float32)
    k = np.asarray(k_packed, dtype=np.float32)
    v = np.asarray(v_packed, dtype=np.float32)
    idx = np.asarray(topk_indices)
    ts = np.asarray(topk_scores, dtype=np.float32)

    nc, names = _get_program()
    logical_maps = _host_inputs(q, k, v, idx, ts)
    in_maps = [{names[key]: arr for key, arr in m.items()} for m in logical_maps]

    res = run_bass_kernel_spmd(nc, in_maps, core_ids=list(range(NCORES)))
    outn = names["out"]
    parts = []
    for c in range(NCORES):
        oc = res.results[c][outn]  # [P, TCH, H*DV]
        parts.append(oc.transpose(1, 0, 2).reshape(TC, H, DV))
    return np.concatenate(parts, axis=0).astype(np.float32)


if __name__ == "__main__":
    rng = np.random.default_rng(0)
    q = rng.standard_normal((T, H, D), dtype=np.float32)
    k = rng.standard_normal((T, H, D), dtype=np.float32)
    v = rng.standard_normal((T, H, DV), dtype=np.float32)
    idx = rng.integers(0, T, size=(T, TK), dtype=np.int64)
    ts = rng.random((T, TK), dtype=np.float32)
    out = kernel(q, k, v, idx, ts)
    print(out.shape, out.dtype)


# revision 8
# speedup vs baseline: 1.5299x; 1.1441x over previous
"""DSA varlen sparse attention for Trainium2, 8 NeuronCores.

Strategy (token-sharded, K/V replicated per core):
  Per core c: tokens t in [c*256, (c+1)*256).
  Instead of gathering 64 K/V rows per token (536 MB of gather traffic),
  compute DENSE per-head scores S^T[j, t] = sum_d K[j,h,d] q[t,h,d] on the
  PE array in bf16, then multiply exp(S^T) by a scattered sparse weight
  matrix tsd^T[j, t] = sum_{k: topk_idx[t,k]=j} topk_scores[t,k]
  (zero elsewhere).  Because softmax's Z cancels in the reference's
  renormalization, the output is exactly
     out[t,h] = (sum_j exp(s[j,t]) * tsd[j,t] * V[j,h]) / (sum_j exp*tsd).
  The sparse scatter runs on-device with GPSIMD local_scatter
  (per-partition scatter, tokens on partitions); duplicate indices are
  pre-merged with a pairwise is_equal/reduce pass on the vector engine.
  The denominator rides as a leading "ones" column of V through the same
  PSUM accumulation.

  Engine schedule: DVE does the dedup chain while PE/ACT run per-head
  S^T matmuls + exp (which don't need the mask); GPSIMD scatters, PE
  transposes tsd, then phase B (mask-mul + AV matmuls + normalize)
  drains per (head, token-chunk).  bf16 inputs are prepared host-side
  (layout/sharding prep); all matmul accumulation is fp32 in PSUM.
"""

import numpy as np
import ml_dtypes
from contextlib import ExitStack

T, H, D, DV, TK = 2048, 8, 128, 128, 64
NCORES = 8
TC = T // NCORES          # 256 tokens per core
P = 128
TCH = TC // P             # 2 token chunks of 128
JC = T // P               # 16 key chunks of 128
SCALE = float(D) ** -0.5
HALF = 1024               # local_scatter num_elems limit is < 2048

_CACHE = {}
SAFE_DEDUP = False  # True: mark duplicate slots -1 (CoreSim asserts uniqueness)


def _build_program(safe_dedup=None):
    if safe_dedup is None:
        safe_dedup = SAFE_DEDUP
    import concourse.mybir as mybir
    import concourse.tile as tile
    from concourse import bacc

    dt = mybir.dt
    Alu = mybir.AluOpType
    Act = mybir.ActivationFunctionType
    Ax = mybir.AxisListType

    nc = bacc.Bacc(None, target_bir_lowering=False, debug=False)
    names = {}
    with ExitStack() as ctx:
        tc = ctx.enter_context(tile.TileContext(nc))
        dram = ctx.enter_context(tc.tile_pool(name="dram", bufs=1, space="DRAM"))
        sb = ctx.enter_context(tc.tile_pool(name="sb", bufs=1))
        pT_pool = ctx.enter_context(tc.tile_pool(name="pTp", bufs=6))
        sm = ctx.enter_context(tc.tile_pool(name="sm", bufs=1))
        sm2 = ctx.enter_context(tc.tile_pool(name="sm2", bufs=2))
        sps = ctx.enter_context(tc.tile_pool(name="spsum", bufs=1, space="PSUM"))
        ops = ctx.enter_context(tc.tile_pool(name="opsum", bufs=4, space="PSUM"))

        # ---------------- DRAM I/O (bf16 data prepped host-side) ----------
        q_d = dram.tile([P, TCH, H * D], dt.bfloat16, kind="ExternalInput")
        k_d = dram.tile([T, H * D], dt.bfloat16, kind="ExternalInput")
        v_d = dram.tile([P, JC * H * (1 + DV)], dt.bfloat16, kind="ExternalInput")
        idx_d = dram.tile([P, TCH, TK], dt.int16, kind="ExternalInput")
        ts_d = dram.tile([P, TCH, TK], dt.bfloat16, kind="ExternalInput")
        ut_d = dram.tile([P, TK * TK], dt.bfloat16, kind="ExternalInput")
        id_d = dram.tile([P, P], dt.bfloat16, kind="ExternalInput")
        out_d = dram.tile([P, TCH, H * DV], dt.float32, kind="ExternalOutput")

        names.update(
            q=q_d.name, k=k_d.name, v=v_d.name, idx=idx_d.name, ts=ts_d.name,
            ut=ut_d.name, ident=id_d.name, out=out_d.name,
        )

        # ---------------- SBUF persistent ----------------
        kT = sb.tile([P, H, T], dt.bfloat16, tag="kT")                 # 32KB/p
        vE = sb.tile([P, JC, H, 1 + DV], dt.bfloat16, tag="vE")        # 33KB/p
        qbf = sb.tile([P, TCH, H * D], dt.bfloat16, tag="qbf")
        qT = sb.tile([P, H, TC], dt.bfloat16, tag="qT")
        tsd = sb.tile([P, TCH, T], dt.bfloat16, tag="tsd")
        tsdT = sb.tile([P, JC, TC], dt.bfloat16, tag="tsdT")
        ut = sb.tile([P, TK * TK], dt.bfloat16, tag="ut")
        ident = sb.tile([P, P], dt.bfloat16, tag="ident")
        idx16 = sb.tile([P, TCH, TK], dt.int16, tag="idx16")
        tsbf = sb.tile([P, TCH, TK], dt.bfloat16, tag="tsbf")
        outs = sb.tile([P, TCH, H * DV], dt.float32, tag="outs")

        # ---------------- loads (small first; all HWDGE) ----------------
        nc.sync.dma_start(out=idx16[:], in_=idx_d[:])
        nc.sync.dma_start(out=tsbf[:], in_=ts_d[:])
        nc.sync.dma_start(out=ident[:], in_=id_d[:])
        # big plain copies ride SWDGE; transposes own the sync HWDGE queue
        # (concurrent HWDGE copy + xbar transpose is a known HW hazard)
        nc.gpsimd.dma_start(out=qbf[:], in_=q_d[:])
        nc.gpsimd.dma_start(out=ut[:], in_=ut_d[:])
        for h in range(H):
            nc.sync.dma_start_transpose(
                out=kT[:, h, :], in_=k_d[:, h * D : (h + 1) * D]
            )
        # V (+ ones column) pre-laid-out on host
        nc.gpsimd.dma_start(
            out=vE[:].rearrange("p a b c -> p (a b c)"), in_=v_d[:]
        )

        # ---------------- dedup chain (DVE), both chunks fused ----------
        # Every slot of a duplicate group receives the same group-sum, so
        # scattering all slots is idempotent -- no last-occurrence masking
        # needed (except for CoreSim, which asserts index uniqueness).
        idxf = sm2.tile([P, TCH, TK], dt.float32, tag="idxf")
        nc.vector.tensor_copy(out=idxf[:], in_=idx16[:])
        eq = sm.tile([P, TCH, TK, TK], dt.bfloat16, tag="eq")
        nc.vector.tensor_tensor(
            out=eq[:],
            in0=idxf[:, :, :, None].to_broadcast([P, TCH, TK, TK]),
            in1=idxf[:, :, None, :].to_broadcast([P, TCH, TK, TK]),
            op=Alu.is_equal,
        )
        # ts'_k = sum_{k'} eq * ts_{k'}  (in-place: eq -> eq*ts, then reduce)
        nc.vector.tensor_tensor(
            out=eq[:], in0=eq[:],
            in1=tsbf[:, :, None, :].to_broadcast([P, TCH, TK, TK]), op=Alu.mult,
        )
        tsum = sm2.tile([P, TCH, TK], dt.bfloat16, tag="tsum")
        with nc.allow_low_precision("duplicate-group sums have <=4 terms"):
            nc.vector.tensor_reduce(out=tsum[:], in_=eq[:], axis=Ax.X, op=Alu.add)

        if safe_dedup:
            # keep only last occurrence: u_k = max_{k'>k} eq2, eq2 recomputed
            eq2 = sm.tile([P, TCH, TK, TK], dt.bfloat16, tag="eq2")
            nc.vector.tensor_tensor(
                out=eq2[:],
                in0=idxf[:, :, :, None].to_broadcast([P, TCH, TK, TK]),
                in1=idxf[:, :, None, :].to_broadcast([P, TCH, TK, TK]),
                op=Alu.is_equal,
            )
            nc.vector.tensor_tensor(
                out=eq2[:], in0=eq2[:],
                in1=ut[:].rearrange("p (a b) -> p a b", a=TK)[:, None, :, :]
                .to_broadcast([P, TCH, TK, TK]),
                op=Alu.mult,
            )
            u = sm2.tile([P, TCH, TK], dt.bfloat16, tag="u")
            nc.vector.tensor_reduce(out=u[:], in_=eq2[:], axis=Ax.X, op=Alu.max)
            keep = sm2.tile([P, TCH, TK], dt.float32, tag="keep")
            nc.vector.tensor_scalar(
                out=keep[:], in0=u[:], scalar1=0.0, scalar2=None, op0=Alu.is_equal
            )
            a = sm2.tile([P, TCH, TK], dt.float32, tag="a")
            nc.vector.tensor_scalar_add(out=a[:], in0=idxf[:], scalar1=1.0)
            b = sm2.tile([P, TCH, TK], dt.float32, tag="b")
            nc.vector.tensor_tensor(out=b[:], in0=a[:], in1=keep[:], op=Alu.mult)
        else:
            # b = idx + 1 for every slot (duplicates scatter the same value)
            b = sm2.tile([P, TCH, TK], dt.float32, tag="b")
            nc.vector.tensor_scalar_add(out=b[:], in0=idxf[:], scalar1=1.0)
        # low half: idx' = b*[b < 1025] - 1  (in [0,1024) or -1)
        mlo = sm2.tile([P, TCH, TK], dt.float32, tag="mlo")
        nc.vector.tensor_scalar(
            out=mlo[:], in0=b[:], scalar1=float(HALF) + 0.5, scalar2=None,
            op0=Alu.is_lt,
        )
        c2 = sm2.tile([P, TCH, TK], dt.float32, tag="c2")
        nc.vector.tensor_tensor(out=c2[:], in0=b[:], in1=mlo[:], op=Alu.mult)
        ilo = sm2.tile([P, TCH, TK], dt.int16, tag="ilo")
        nc.vector.tensor_scalar_add(out=ilo[:], in0=c2[:], scalar1=-1.0)
        # high half: idx' = (b - 1024)*[b > 1024] - 1
        mhi = sm2.tile([P, TCH, TK], dt.float32, tag="mhi")
        nc.vector.tensor_scalar(
            out=mhi[:], in0=b[:], scalar1=float(HALF) + 0.5, scalar2=None,
            op0=Alu.is_gt,
        )
        d1 = sm2.tile([P, TCH, TK], dt.float32, tag="d1")
        nc.vector.tensor_scalar_add(out=d1[:], in0=b[:], scalar1=-float(HALF))
        d2 = sm2.tile([P, TCH, TK], dt.float32, tag="d2")
        nc.vector.tensor_tensor(out=d2[:], in0=d1[:], in1=mhi[:], op=Alu.mult)
        ihi = sm2.tile([P, TCH, TK], dt.int16, tag="ihi")
        nc.vector.tensor_scalar_add(out=ihi[:], in0=d2[:], scalar1=-1.0)

        for t in range(TCH):
            nc.gpsimd.local_scatter(
                out_ap=tsd[:, t, 0:HALF], data_ap=tsum[:, t, :], idxs_ap=ilo[:, t, :],
                channels=P, num_elems=HALF, num_idxs=TK,
            )
            nc.gpsimd.local_scatter(
                out_ap=tsd[:, t, HALF:T], data_ap=tsum[:, t, :], idxs_ap=ihi[:, t, :],
                channels=P, num_elems=HALF, num_idxs=TK,
            )

        # ---------------- q transposes: qT[d, h, t] (PE, early) ----------
        for h in range(H):
            for t in range(TCH):
                ps = ops.tile([P, P], dt.bfloat16, tag="op")
                nc.tensor.transpose(
                    out=ps[:], in_=qbf[:, t, h * D : (h + 1) * D], identity=ident[:]
                )
                nc.scalar.copy(out=qT[:, h, t * P : (t + 1) * P], in_=ps[:])

        # ---------------- phase A: S^T + exp for every head ----------------
        G = 8  # score chunks per PSUM tile (4 banks)
        pTs = []
        for h in range(H):
            pT = pT_pool.tile([P, JC, TC], dt.bfloat16, tag="pT")
            pTs.append(pT)
            for g in range(JC // G):
                sp = sps.tile([P, G, TC], dt.float32, tag="sp")
                for j in range(G):
                    jc = g * G + j
                    nc.tensor.matmul(
                        out=sp[:, j, :],
                        lhsT=kT[:, h, jc * P : (jc + 1) * P],
                        rhs=qT[:, h, :],
                        start=True, stop=True,
                    )
                nc.scalar.activation(
                    out=pT[:, g * G : (g + 1) * G, :], in_=sp[:],
                    func=Act.Exp, scale=SCALE,
                )

        # ------------- tsd transpose: tsdT[j, t] (PE after phase A) ------
        # chunk-0 drains on ACT, chunk-1 drains on DVE, so phase B's
        # t=0 work can start while chunk 1 still drains.
        for t in range(TCH):
            for jc in range(JC):
                ps = ops.tile([P, P], dt.bfloat16, tag="op")
                nc.tensor.transpose(
                    out=ps[:], in_=tsd[:, t, jc * P : (jc + 1) * P], identity=ident[:]
                )
                dst = tsdT[:, jc, t * P : (t + 1) * P]
                if t == 0:
                    nc.scalar.copy(out=dst, in_=ps[:])
                else:
                    nc.vector.tensor_copy(out=dst, in_=ps[:])

        # ---------------- phase B: mask + AV + normalize ----------------
        for h in range(H):
            pT = pTs[h]
            for t in range(TCH):
                nc.vector.tensor_tensor(
                    out=pT[:, :, t * P : (t + 1) * P],
                    in0=pT[:, :, t * P : (t + 1) * P],
                    in1=tsdT[:, :, t * P : (t + 1) * P],
                    op=Alu.mult,
                )
                op = ops.tile([P, 1 + DV], dt.float32, tag="op")
                for jc in range(JC):
                    nc.tensor.matmul(
                        out=op[:],
                        lhsT=pT[:, jc, t * P : (t + 1) * P],
                        rhs=vE[:, jc, h, :],
                        start=(jc == 0), stop=(jc == JC - 1),
                    )
                rec = sm2.tile([P, 1], dt.float32, tag="rec")
                nc.vector.reciprocal(out=rec[:], in_=op[:, 0:1])
                nc.scalar.mul(
                    out=outs[:, t, h * DV : (h + 1) * DV], in_=op[:, 1 : 1 + DV],
                    mul=rec[:],
                )

        for t in range(TCH):
            nc.sync.dma_start(out=out_d[:, t, :], in_=outs[:, t, :])

    nc.compile()
    return nc, names


def _get_program():
    key = ("prog", SAFE_DEDUP)
    if key not in _CACHE:
        _CACHE[key] = _build_program()
    return _CACHE[key]


def _host_inputs(q, k, v, idx, ts):
    """Build per-core in_maps (host-side shard/layout/dtype prep)."""
    bf16 = ml_dtypes.bfloat16
    # strict upper-triangle (k' > k) replicated over partitions
    utm = np.triu(np.ones((TK, TK), np.float32), 1).reshape(1, TK * TK)
    utm = np.ascontiguousarray(np.broadcast_to(utm, (P, TK * TK))).astype(bf16)
    identity = np.eye(P, dtype=np.float32).astype(bf16)

    k_full = np.ascontiguousarray(k.reshape(T, H * D)).astype(bf16)
    # vE[p, jc, h, 0] = 1, vE[p, jc, h, 1:] = V[jc*128+p, h, :]
    v_r = v.reshape(JC, P, H, DV).transpose(1, 0, 2, 3)  # [P, JC, H, DV]
    v_full = np.ones((P, JC, H, 1 + DV), dtype=np.float32)
    v_full[:, :, :, 1:] = v_r
    v_full = v_full.reshape(P, JC * H * (1 + DV)).astype(bf16)

    maps = []
    for c in range(NCORES):
        sl = slice(c * TC, (c + 1) * TC)
        qc = (
            q[sl].reshape(TCH, P, H * D).transpose(1, 0, 2)
        )  # [P, TCH, H*D], t = tc*128 + p
        ic = idx[sl].astype(np.int16).reshape(TCH, P, TK).transpose(1, 0, 2)
        tc_ = ts[sl].reshape(TCH, P, TK).transpose(1, 0, 2)
        maps.append(
            dict(
                q=np.ascontiguousarray(qc).astype(bf16),
                k=k_full,
                v=v_full,
                idx=np.ascontiguousarray(ic),
                ts=np.ascontiguousarray(tc_).astype(bf16),
                ut=utm,
                ident=identity,
            )
        )
    return maps


def kernel(q_packed, k_packed, v_packed, topk_indices, topk_scores):
    from concourse.bass_utils import run_bass_kernel_spmd

    q = np.asarray(q_packed, dtype=np.float32)
    k = np.asarray(k_packed, dtype=np.float32)
    v = np.asarray(v_packed, dtype=np.float32)
    idx = np.asarray(topk_indices)
    ts = np.asarray(topk_scores, dtype=np.float32)

    nc, names = _get_program()
    logical_maps = _host_inputs(q, k, v, idx, ts)
    in_maps = [{names[key]: arr for key, arr in m.items()} for m in logical_maps]

    res = run_bass_kernel_spmd(nc, in_maps, core_ids=list(range(NCORES)))
    outn = names["out"]
    parts = []
    for c in range(NCORES):
        oc = res.results[c][outn]  # [P, TCH, H*DV]
        parts.append(oc.transpose(1, 0, 2).reshape(TC, H, DV))
    return np.concatenate(parts, axis=0).astype(np.float32)


if __name__ == "__main__":
    rng = np.random.default_rng(0)
    q = rng.standard_normal((T, H, D), dtype=np.float32)
    k = rng.standard_normal((T, H, D), dtype=np.float32)
    v = rng.standard_normal((T, H, DV), dtype=np.float32)
    idx = rng.integers(0, T, size=(T, TK), dtype=np.int64)
    ts = rng.random((T, TK), dtype=np.float32)
    out = kernel(q, k, v, idx, ts)
    print(out.shape, out.dtype)
